# revision 39
# baseline (speedup 1.0000x reference)
"""Longformer attention Bass kernel for 8 TRN2 NeuronCores (v3, bf16).

Sharding: core c handles batch b = c//4 and heads 4*(c%4) .. 4*(c%4)+3.
Each core computes its 4 heads' attention + the partial output projection;
the host sums the 4 partials per batch element and adds the folded bias.

Design notes:
- all matmul operands bf16 (PSUM accumulates fp32); fp8 was tried and
  rejected: e4m3/e5m2 element-wise quantization error (~2.6-5%) passes
  through the mean-zero dot products undamped, blowing the 2e-2 budget
- scores computed transposed (s^T [keys, queries]) so softmax needs no
  transposes: denominator comes from a ones-column appended to V and the
  per-query reciprocal is partition-broadcast on GpSimd
- tight piece plan: c=0/1 (global keys) full width; c>=2 one 208-wide
  global-query piece (half 0) + an arbitrary-offset ~256-wide window
  piece, exact-cover validated against the reference mask
- attention runs as 8 single-head passes so live ctx PSUM drops to 2
  banks, letting the score pool go to 5 bufs (rotation depth > the exp
  round-trip latency that stalled 2-head grouping)
- adjacent small pieces pair into one PSUM bank (bases 0/256) and share
  a single exp instruction, halving Scalar's per-instruction overhead
- three-group PV lag + projection/output chains as fillers keep the PE
  fed; den/recip/ctx-scale on Vector, broadcast on GpSimd
"""

import os
import numpy as np
import ml_dtypes

import concourse.bass as bass
import concourse.mybir as mybir
import concourse.tile as tile
from concourse import bacc
from concourse.bass_utils import run_bass_kernel_spmd

# ---- problem constants (hardcoded per contract) ----
B, S, DM = 2, 2048, 1024
H, DH = 16, 64
WINDOW = 128
NG = max(1, int(S * 0.1))  # 204 global tokens
SCALE = 1.0 / np.sqrt(DH)
NCORES = 8
HPC = 4            # heads per core
F = HPC * DH       # 256 per-core head features
KB = S // 128      # 16 key blocks

FP = mybir.dt.float32
BF = mybir.dt.bfloat16
F8E4 = mybir.dt.float8e4
F8E5 = mybir.dt.float8e5
DR = mybir.MatmulPerfMode.DoubleRow
AF = mybir.ActivationFunctionType
BF_NP = ml_dtypes.bfloat16
E4_NP = ml_dtypes.float8_e4m3
E5_NP = ml_dtypes.float8_e5m2


# ---------------------------------------------------------------- planning
def _allow():
    pos = np.arange(S)
    dist = pos[None, :] - pos[:, None]
    window = np.abs(dist) <= WINDOW // 2
    isg = pos < NG
    return window | isg[:, None] | isg[None, :]  # [query i, key j]


def _keyset(g):
    if g == 0:
        return list(range(KB))
    s = {0, 1}
    for c in range(2 * g - 1, 2 * g + 3):
        if 0 <= c < KB:
            s.add(c)
    return sorted(s)


def _plan3():
    """Tight per-half piece plan.

    c=0/1 (global keys) get full-width pieces; c>=2 gets a global-query
    piece (half 0 only, queries 0..208) plus an arbitrary-offset window
    piece clipped to the half.  Trailing all-masked key rows are trimmed
    (k0 stays 0 so matmul base partitions match).

    item: dict(c, w, q0, k0, k1, ops, pvs)
      ops: ('mul', mask_idx, o, ow) or ('memset', r0, r1, c0, c1)
      pvs: [(o, ln, lp, off, start, stop)]  lp = absolute 512-query block
    """
    allowT = _allow().T  # [key, query]
    masks, midx = [], {}

    def mask_id(sub):
        key = (sub.shape, sub.tobytes())
        if key not in midx:
            pad = np.zeros((128, 256), np.float32)
            pad[:sub.shape[0], :sub.shape[1]] = sub
            midx[key] = len(masks)
            masks.append(pad)
        return midx[key]

    halves = []
    for half in (0, 1):
        Q0 = 1024 * half
        items = []

        def add_piece(c, q0, w):
            sub = allowT[c * 128:(c + 1) * 128, q0:q0 + w]
            rows = np.nonzero(sub.any(axis=1))[0]
            k1 = int(rows.max()) + 1
            sub = sub[:k1]
            ops = []
            for o in range(0, w, 256):
                ow = min(256, w - o)
                ss = sub[:, o:o + ow]
                if ss.all():
                    continue
                rfull = ss.all(axis=1)
                rnone = ~ss.any(axis=1)
                cfull = ss.all(axis=0)
                cnone = ~ss.any(axis=0)
                done = False
                if (rfull | rnone).all() and rnone.any():
                    idx = np.nonzero(rnone)[0]
                    r0, r1 = int(idx.min()), int(idx.max()) + 1
                    if rnone[r0:r1].all() and r1 - r0 == len(idx) \
                            and r0 % 32 == 0 and (r1 % 32 == 0 or r1 == k1):
                        ops.append(("memset", r0, r1, o, o + ow))
                        done = True
                if not done and (cfull | cnone).all() and cnone.any():
                    idx = np.nonzero(cnone)[0]
                    c0, c1 = int(idx.min()), int(idx.max()) + 1
                    if cnone[c0:c1].all() and c1 - c0 == len(idx):
                        ops.append(("memset", 0, k1, o + c0, o + c1))
                        done = True
                if not done:
                    ops.append(("mul", mask_id(ss), o, ow))
            segs = []
            o = 0
            while o < w:
                qa = q0 + o
                ln = min(w - o, 512 - (qa % 512))
                segs.append([o, ln, qa // 512, qa % 512])
                o += ln
            items.append(dict(c=c, w=w, q0=q0, k0=0, k1=k1, ops=ops,
                              segs=segs))

        add_piece(0, Q0, 512)
        add_piece(0, Q0 + 512, 512)
        add_piece(1, Q0, 512)
        add_piece(1, Q0 + 512, 512)
        for c in range(2, KB):
            if half == 0:
                add_piece(c, 0, 208)
            lo = max(128 * c - 64, Q0 + (208 if half == 0 else 0))
            hi = min(128 * c + 192, Q0 + 1024)
            if hi > lo:
                add_piece(c, lo, hi - lo)

        first_seen, last_seen = {}, {}
        for i, it in enumerate(items):
            for sg in it["segs"]:
                lp = sg[2]
                if lp not in first_seen:
                    first_seen[lp] = (i, sg[0])
                last_seen[lp] = (i, sg[0])
        for i, it in enumerate(items):
            pvs = []
            for sg in it["segs"]:
                o, ln, lp, off = sg
                pvs.append((o, ln, lp, off,
                            first_seen[lp] == (i, o),
                            last_seen[lp] == (i, o)))
            it["pvs"] = pvs
            del it["segs"]
        halves.append(items)
    return halves, np.stack(masks)


# ---------------------------------------------------------------- builder
def _emit(tc, halves, nm, aps):
    nc = tc.nc
    hT, wqT, wkT, wvT, woT, bq2, bk2, mks, out = aps

    with tc.tile_pool(name="const", bufs=1) as const:
        hT_sb = const.tile([128, 8, S], BF, tag="hT")
        wq_sb = const.tile([128, 8, F], BF, tag="wq")
        wk_sb = const.tile([128, 8, F], BF, tag="wk")
        wv_sb = const.tile([128, 8, F], BF, tag="wv")
        wo_sb = const.tile([128, 2, DM], BF, tag="wo")
        bq_sb = const.tile([128, 2], FP, tag="bq")
        bk_sb = const.tile([128, 2], FP, tag="bk")
        mk_sb = const.tile([128, nm, 256], BF, tag="mk")
        qT_sb = const.tile([128, 2, S], BF, tag="qT")
        kT_sb = const.tile([128, 2, S], BF, tag="kT")
        v_sb = const.tile([128, HPC, KB, DH + 1], BF, tag="v")
        cT_sb = const.tile([128, 2, S], BF, tag="cT")

        # wq + the first hT quarter first so the first projection chain
        # can start early; the rest of hT streams nt-major behind it
        hT_r = hT.rearrange("(t p) n -> p t n", p=128)
        nc.sync.dma_start(wq_sb[:], wqT.rearrange("(t p) f -> p t f", p=128))
        for kt in range(8):
            nc.sync.dma_start(hT_sb[:, kt, 0:512], hT_r[:, kt, 0:512])
        nc.sync.dma_start(wk_sb[:], wkT.rearrange("(t p) f -> p t f", p=128))
        nc.sync.dma_start(wv_sb[:], wvT.rearrange("(t p) f -> p t f", p=128))
        nc.sync.dma_start(bq_sb[:], bq2.rearrange("t p -> p t"))
        nc.sync.dma_start(bk_sb[:], bk2.rearrange("t p -> p t"))
        for ntq in range(1, 4):
            nc.sync.dma_start(
                hT_sb[:, :, ntq * 512:(ntq + 1) * 512],
                hT_r[:, :, ntq * 512:(ntq + 1) * 512])
        nc.sync.dma_start(wo_sb[:], woT.rearrange("(t p) f -> p t f", p=128))
        nc.sync.dma_start(mk_sb[:], mks.rearrange("n p m -> p n m"))
        nc.vector.memset(v_sb[:, :, :, DH:DH + 1], 1.0)

        with tc.tile_pool(name="ps", bufs=5, space="PSUM") as psp, \
             tc.tile_pool(name="pctx", bufs=3, space="PSUM") as pctx, \
             tc.tile_pool(name="work", bufs=8) as work, \
             tc.tile_pool(name="rcb", bufs=2) as rcbp, \
             tc.tile_pool(name="rcp", bufs=2) as rcp, \
             tc.tile_pool(name="ostg", bufs=4) as ostg:

            def chain(kind, *args):
                """Deferred projection / output chains, all on one pool."""
                ps = psp.tile([128, 512], FP, tag="sT",
                              name=f"ch_{kind}_{args}")
                if kind in ("q", "k"):
                    mt, nt = args
                    wsb, bsb, dst = ((wq_sb, bq_sb, qT_sb) if kind == "q"
                                     else (wk_sb, bk_sb, kT_sb))
                    for kt in range(8):
                        nc.tensor.matmul(
                            ps[:], wsb[:, kt, mt * 128:(mt + 1) * 128],
                            hT_sb[:, kt, nt * 512:(nt + 1) * 512],
                            start=(kt == 0), stop=(kt == 7))
                    nc.vector.tensor_scalar_add(
                        dst[:, mt, nt * 512:(nt + 1) * 512], ps[:],
                        bsb[:, mt:mt + 1])
                elif kind == "v":
                    st = args[0]  # covers st, st+1
                    for sub in range(2):
                        for kt in range(8):
                            nc.tensor.matmul(
                                ps[:, sub * 256:(sub + 1) * 256],
                                hT_sb[:, kt, (st + sub) * 128:
                                      (st + sub + 1) * 128],
                                wv_sb[:, kt, :],
                                start=(kt == 0), stop=(kt == 7))
                    nc.vector.tensor_copy(
                        v_sb[:, :, st:st + 2, 0:DH],
                        ps.rearrange("p (c h d) -> p h c d", c=2, h=HPC))
                else:  # "po"
                    st, nt = args
                    for kt in range(2):
                        nc.tensor.matmul(
                            ps[:], cT_sb[:, kt, st * 128:(st + 1) * 128],
                            wo_sb[:, kt, nt * 512:(nt + 1) * 512],
                            start=(kt == 0), stop=(kt == 1))
                    ot = ostg.tile([128, 512], BF, tag="ot",
                                   name=f"ot{st}_{nt}")
                    nc.scalar.activation(ot[:], ps[:], AF.Copy)
                    nc.sync.dma_start(
                        out[st * 128:(st + 1) * 128,
                            nt * 512:(nt + 1) * 512], ot[:])

            # minimal prefix so (half0, head 0) can start
            for ck in (("q", 0, 0), ("q", 0, 1), ("k", 0, 0),
                       ("v", 0), ("v", 2)):
                chain(*ck)

            fillers = {
                (0, 0): [("k", 0, 1), ("v", 4), ("k", 0, 2), ("v", 6),
                         ("k", 0, 3), ("v", 8), ("v", 10), ("v", 12),
                         ("v", 14)],
                (0, 1): [("q", 1, 0), ("q", 1, 1), ("k", 1, 0),
                         ("k", 1, 1)],
                (0, 2): [("k", 1, 2), ("k", 1, 3), ("q", 1, 2),
                         ("q", 0, 2)],
                (0, 3): [("q", 1, 3), ("q", 0, 3)],
                (1, 0): [("po", st, nt) for st in range(3)
                         for nt in range(2)],
                (1, 1): [("po", st, nt) for st in range(3, 6)
                         for nt in range(2)],
                (1, 2): [("po", st, nt) for st in range(6, 8)
                         for nt in range(2)],
                (1, 3): [],
            }
            # emitted after a pass's full PV flush (cT for the half's own
            # early st blocks is complete once every head's ctx is scaled)
            post = {
                (1, 3): [("po", st, nt) for st in range(8, 10)
                         for nt in range(2)],
            }

            for half in (0, 1):
                items = halves[half]
                # group adjacent small pieces (w <= 256) in pairs sharing
                # one PSUM bank at column bases 0 / 256 -> one exp per pair
                groups = []
                i = 0
                while i < len(items):
                    if (i + 1 < len(items) and items[i]["w"] <= 256
                            and items[i + 1]["w"] <= 256):
                        groups.append([(i, 0), (i + 1, 256)])
                        i += 2
                    else:
                        groups.append([(i, 0)])
                        i += 1
                pair_done = {}
                for gi, grp in enumerate(groups):
                    for i, _ in grp:
                        for pv in items[i]["pvs"]:
                            pair_done[pv[2]] = gi
                for hp in (0, 1, 2, 3):
                    heads = (hp,)
                    fq = list(fillers[(half, hp)])
                    ctx = {}
                    for h in heads:
                        for lp in sorted({pv[2] for it in items
                                          for pv in it["pvs"]}):
                            ctx[(h, lp)] = pctx.tile(
                                [DH + 1, 512], FP, tag="ctx",
                                name=f"ctx{half}_{h}_{lp}")
                    exs = {}

                    def pv_block(gi):
                        for i, base in groups[gi]:
                            it = items[i]
                            c, k0, k1 = it["c"], it["k0"], it["k1"]
                            for h in heads:
                                ex = exs[(h, gi)]
                                for (o, ln, lp, off, st_, sp_) in it["pvs"]:
                                    nc.tensor.matmul(
                                        ctx[(h, lp)][:, off:off + ln],
                                        v_sb[k0:k1, h, c, :],
                                        ex[0:k1 - k0, base + o:base + o + ln],
                                        start=st_, stop=sp_,
                                        skip_group_check=True)
                        for h in heads:
                            exs.pop((h, gi))
                        for lp, di in pair_done.items():
                            if di != gi:
                                continue
                            for h in heads:
                                p0, mt = 64 * (h % 2), h // 2
                                den = rcp.tile([1, 512], FP, tag="den",
                                               name=f"den{half}_{h}_{lp}")
                                nc.vector.tensor_copy(
                                    den[:], ctx[(h, lp)][DH:DH + 1, :])
                                rc = rcp.tile([1, 512], FP, tag="rc",
                                              name=f"rc{half}_{h}_{lp}")
                                nc.vector.reciprocal_approx_fast(rc[:],
                                                                 den[:])
                                rcb = rcbp.tile([DH, 512], FP, tag="rcb",
                                                name=f"rcb{half}_{h}_{lp}")
                                nc.gpsimd.partition_broadcast(rcb[:], rc[:],
                                                              channels=DH)
                                dst = cT_sb[p0:p0 + 64, mt,
                                            lp * 512:(lp + 1) * 512]
                                nc.vector.tensor_mul(
                                    dst, ctx[(h, lp)][0:DH, :], rcb[:])

                    pending = []
                    for gi, grp in enumerate(groups):
                        kmax = max(items[i]["k1"] - items[i]["k0"]
                                   for i, _ in grp)
                        wtot = grp[-1][1] + items[grp[-1][0]]["w"]
                        for h in heads:
                            p0, mt = 64 * (h % 2), h // 2
                            ps = psp.tile([128, 512], FP, tag="sT",
                                          name=f"sT{half}_{h}_{gi}")
                            ex = work.tile([128, 512], BF, tag="ex",
                                           name=f"ex{half}_{h}_{gi}")
                            for i, base in grp:
                                it = items[i]
                                c, w, q0 = it["c"], it["w"], it["q0"]
                                k0, k1 = it["k0"], it["k1"]
                                nc.tensor.matmul(
                                    ps[0:k1 - k0, base:base + w],
                                    kT_sb[p0:p0 + 64, mt,
                                          c * 128 + k0:c * 128 + k1],
                                    qT_sb[p0:p0 + 64, mt, q0:q0 + w],
                                    start=True, stop=True,
                                    skip_group_check=True)
                            nc.scalar.activation(ex[0:kmax, 0:wtot],
                                                 ps[0:kmax, 0:wtot], AF.Exp)
                            for i, base in grp:
                                it = items[i]
                                km = it["k1"] - it["k0"]
                                for op in it["ops"]:
                                    if op[0] == "mul":
                                        _, mi, o, ow = op
                                        nc.vector.tensor_mul(
                                            ex[0:km, base + o:base + o + ow],
                                            ex[0:km, base + o:base + o + ow],
                                            mk_sb[0:km, mi, 0:ow])
                                    else:
                                        _, r0, r1, c0, c1 = op
                                        nc.vector.memset(
                                            ex[r0:r1, base + c0:base + c1],
                                            0.0)
                            exs[(h, gi)] = ex
                        if len(pending) >= 3:
                            pv_block(pending.pop(0))
                        if fq:
                            chain(*fq.pop(0))
                        pending.append(gi)
                    for p in pending:
                        pv_block(p)
                    while fq:
                        chain(*fq.pop(0))
                    for ck in post.get((half, hp), ()):
                        chain(*ck)

            # ---- phase 3 tail: remaining output projection
            for st in range(10, KB):
                for nt in range(2):
                    chain("po", st, nt)


_CACHE = {}
TRACE_KWARGS = {}  # test harness may set e.g. dict(tmpdir=...)


def _get_nc():
    if "nc" in _CACHE:
        return _CACHE["nc"], _CACHE["masks"]
    halves, masks = _plan3()
    nm = masks.shape[0]
    nc = bacc.Bacc("TRN2", target_bir_lowering=False, debug=False,
                   enable_asserts=False)

    def dp(name, shape, dtype=BF, is_out=False):
        h = nc.declare_dram_parameter(name, list(shape), dtype, isOutput=is_out)
        return h[:]

    aps = (
        dp("hT", [DM, S]),
        dp("wqT", [DM, F]),
        dp("wkT", [DM, F]),
        dp("wvT", [DM, F]),
        dp("woT", [F, DM]),
        dp("bq2", [2, 128], FP),
        dp("bk2", [2, 128], FP),
        dp("mks", [nm, 128, 256]),
        dp("out", [S, DM], BF, True),
    )
    with tile.TileContext(nc) as tc:
        _emit(tc, halves, nm, aps)
    nc.compile()
    _CACHE["nc"] = nc
    _CACHE["masks"] = masks
    return nc, masks


def make_in_maps(hidden_states, Wq, bq, Wk, bk, Wv, bv, Wo, bo, masks):
    in_maps = []
    f32 = np.float32
    mks_bf = masks.astype(BF_NP)
    for core in range(NCORES):
        b, fs = core // 4, (core % 4) * F
        in_maps.append({
            "hT": np.ascontiguousarray(hidden_states[b].T).astype(BF_NP),
            "wqT": np.ascontiguousarray((Wq[fs:fs + F] * SCALE).T).astype(BF_NP),
            "wkT": np.ascontiguousarray(Wk[fs:fs + F].T).astype(BF_NP),
            "wvT": np.ascontiguousarray(Wv[fs:fs + F].T).astype(BF_NP),
            "woT": np.ascontiguousarray(Wo[:, fs:fs + F].T).astype(BF_NP),
            "bq2": (bq[fs:fs + F] * SCALE).reshape(2, 128).astype(f32),
            "bk2": bk[fs:fs + F].reshape(2, 128).astype(f32),
            "mks": mks_bf,
        })
    return in_maps


def kernel(hidden_states, Wq, bq, Wk, bk, Wv, bv, Wo, bo):
    nc, masks = _get_nc()
    in_maps = make_in_maps(hidden_states, Wq, bq, Wk, bk, Wv, bv, Wo, bo,
                           masks)
    trace = bool(int(os.environ.get("ATTN_TRACE", "0")))
    kw = dict(TRACE_KWARGS) if trace else {}
    res = run_bass_kernel_spmd(nc, in_maps, core_ids=list(range(NCORES)),
                               trace=trace, **kw)
    _CACHE["last_results"] = res
    bias = (bo + Wo @ bv).astype(np.float32)
    out = np.empty((B, S, DM), np.float32)
    for b in range(B):
        acc = res.results[4 * b]["out"].astype(np.float32).copy()
        for c in range(4 * b + 1, 4 * b + 4):
            acc += res.results[c]["out"]
        out[b] = acc + bias
    return out

